# revision 23
# baseline (speedup 1.0000x reference)
"""CascadeGDCN (3-hop graph diffusion convolution) on 8 Trainium2 NeuronCores.

v4 design:
  - Destination nodes sharded across 8 cores (12544 rows each); edges
    partitioned by destination core; full X replicated per-core in DRAM and
    rebuilt by an AllGather after each hop (skipped after the last hop).
  - Packed-call edge layout: per gather call (4-dest-group block x source
    chunk) the 4 groups' edge segments are packed back-to-back at LANE
    granularity (per-group length = max edge count over the 8 cores, so the
    SPMD program structure is shared); slots of 128 edges may straddle a
    group boundary.  Boundary slots get two one-hot S columns (one per
    group).  This removes most of the per-(group,chunk) ceil padding that a
    slot-aligned layout pays -> ~14% fewer gather descriptors.
  - The gather (Q7 SWDGE descriptor generation at ~2.3 ns/row across 4
    queues) is the kernel bottleneck; idx tables are SBUF-resident and 8
    message buffers keep the gather queues saturated.
  - bf16 compute: S one-hot built by one DVE is_equal per call (vs resident
    denc), edge values folded into messages by one DVE multiply+cast,
    matmuls bf16 (FWL weight load + 1-pass streaming), PSUM accumulates
    [128 dests, 64 feat] per group.
  - st accumulates fp32 in SBUF; new-X rows go psum -> SBUF (ScalarE) ->
    per-group DMA into the bounce buffer feeding the AllGather.
  - The final stage (transpose, Theta matmul, sigmoid, +H) is emitted
    per-block inside the last SpMM so it overlaps the tail of the gathers.
"""

import numpy as np

D = 64
NCORES = 8
NUM_HOPS = 3
N_NODES = 100000
SHARD = 12544            # dest rows per core (98 groups of 128)
NODES_PAD = SHARD * NCORES   # 100352
NCHUNKS = 4
CHUNK = NODES_PAD // NCHUNKS  # 25088 (< 32768 so chunk-local idx fits int16)
GROUPS = SHARD // 128    # 98
GPB = 4                  # dest groups per block (per gather call)
BUFS = 8                 # message-tile pool depth (gather pipelining)


def _softmax(x):
    e = np.exp(x - x.max())
    return e / e.sum()


def _blocks():
    out = []
    g = 0
    while g < GROUPS:
        out.append((g, min(GPB, GROUPS - g)))
        g += GPB
    return out


def _direction_layout(dest, src):
    """Shared (SPMD) packed-call layout for one direction.

    Returns dict with:
      maxc[g, c]      per-cell max edge count over cores
      seg[(g, c)]     lane offset of group g's segment inside call (b, c)
      nreal[(b, c)]   real slots per call;  Oreal[(b, c)] global real offset
      next_[(b, c)]   ext (S) slots per call; Oext[(b, c)] global ext offset
      ents[(b, c)]    list of (slot_local, g, lane_lo, lane_hi) ext entries
      sched[g]        list of (c, ext_local, real_local) matmuls for group g
      nreal_tot, next_tot
    """
    counts = np.zeros((NCORES, GROUPS, NCHUNKS), dtype=np.int64)
    core = dest // SHARD
    for m in range(NCORES):
        sel = core == m
        d_loc = dest[sel] - m * SHARD
        g = d_loc >> 7
        c = src[sel] // CHUNK
        np.add.at(counts, (m, g, c), 1)
    maxc = np.max(counts, axis=0)

    blocks = _blocks()
    seg = {}
    nreal = {}
    next_ = {}
    Oreal = {}
    Oext = {}
    ents = {}
    sched = {g: [] for g in range(GROUPS)}
    pr = 0
    pe = 0
    for bi, (g0, gc) in enumerate(blocks):
        for c in range(NCHUNKS):
            lane = 0
            lo_hi = []
            for gl in range(gc):
                g = g0 + gl
                seg[(g, c)] = lane
                lo_hi.append((g, lane, lane + int(maxc[g, c])))
                lane += int(maxc[g, c])
            ns = (lane + 127) // 128
            nreal[(bi, c)] = ns
            Oreal[(bi, c)] = pr
            pr += ns
            # ext entries: per slot, per overlapping group
            Oext[(bi, c)] = pe
            el = []
            for s in range(ns):
                s_lo, s_hi = s * 128, (s + 1) * 128
                for g, a, b in lo_hi:
                    lo = max(s_lo, a)
                    hi = min(s_hi, b)
                    if lo < hi:
                        el.append((s, g, lo - s_lo, hi - s_lo))
                        sched[g].append((c, len(el) - 1 + pe - Oext[(bi, c)],
                                         s))
            ents[(bi, c)] = el
            next_[(bi, c)] = len(el)
            pe += len(el)
    return {"maxc": maxc, "seg": seg, "nreal": nreal, "next": next_,
            "Oreal": Oreal, "Oext": Oext, "ents": ents, "sched": sched,
            "nreal_tot": pr, "next_tot": pe, "blocks": blocks}


def _prep_direction(dest, src, val, lay):
    """Per-core idx/val (real-slot stream) and denc (ext stream) tables."""
    maxc = lay["maxc"]
    seg = lay["seg"]
    nreal = lay["nreal"]
    Oreal = lay["Oreal"]
    Oext = lay["Oext"]
    ents = lay["ents"]
    blocks = lay["blocks"]
    tot_r = lay["nreal_tot"] * 128
    tot_e = lay["next_tot"]

    # per-cell global lane base = call real base*128 + segment offset
    cell_base = np.zeros(GROUPS * NCHUNKS, dtype=np.int64)
    for g in range(GROUPS):
        bi = g // GPB
        for c in range(NCHUNKS):
            cell_base[g * NCHUNKS + c] = Oreal[(bi, c)] * 128 + seg[(g, c)]

    core = dest // SHARD
    out = []
    for m in range(NCORES):
        sel = core == m
        d_loc = (dest[sel] - m * SHARD).astype(np.int64)
        s = src[sel].astype(np.int64)
        v = val[sel].astype(np.float32)
        g = d_loc >> 7
        c = s // CHUNK
        cell = g * NCHUNKS + c
        order = np.argsort(cell, kind="stable")
        cell_s = cell[order]
        counts = np.bincount(cell_s, minlength=GROUPS * NCHUNKS)
        starts = np.zeros(GROUPS * NCHUNKS, dtype=np.int64)
        starts[1:] = np.cumsum(counts)[:-1]
        rank = np.arange(cell_s.size) - starts[cell_s]
        pos = cell_base[cell_s] + rank

        idx_st = np.zeros(tot_r, dtype=np.int16)
        denc_lane = np.full(tot_r, -1.0, dtype=np.float32)
        val_st = np.zeros(tot_r, dtype=np.float32)
        idx_st[pos] = (s[order] - c[order] * CHUNK).astype(np.int16)
        denc_lane[pos] = (d_loc[order] & 127).astype(np.float32)
        val_st[pos] = v[order]

        # ext denc stream: per ext entry, group lanes only, -1 elsewhere
        denc_ext = np.full((tot_e, 128), -1.0, dtype=np.float32)
        for bi, (g0, gc) in enumerate(blocks):
            for c in range(NCHUNKS):
                ob = Oreal[(bi, c)] * 128
                oe = Oext[(bi, c)]
                for k, (sl, g, lo, hi) in enumerate(ents[(bi, c)]):
                    denc_ext[oe + k, lo:hi] = denc_lane[
                        ob + sl * 128 + lo: ob + sl * 128 + hi]

        idx_tbl = np.tile(np.ascontiguousarray(idx_st.reshape(-1, 16).T),
                          (8, 1))
        denc_tbl = np.ascontiguousarray(denc_ext.T)
        val_tbl = np.ascontiguousarray(val_st.reshape(-1, 128).T)
        out.append({"idx": idx_tbl, "denc": denc_tbl, "val": val_tbl})
    return out


def prep_host(H_l, edge_row, edge_col, edge_val, out_degree, in_degree,
              hop_attention, theta_out, theta_in, Theta):
    from ml_dtypes import bfloat16

    H = np.asarray(H_l, dtype=np.float32)
    er = np.asarray(edge_row, dtype=np.int64)
    ec = np.asarray(edge_col, dtype=np.int64)
    ev = np.asarray(edge_val, dtype=np.float32)
    od = np.asarray(out_degree, dtype=np.float32)
    idg = np.asarray(in_degree, dtype=np.float32)

    alpha = _softmax(np.asarray(hop_attention, dtype=np.float64))
    th_o = np.asarray(theta_out, dtype=np.float64)
    th_i = np.asarray(theta_in, dtype=np.float64)
    coef = [(float(alpha[k] * th_o[k]), float(alpha[k] * th_i[k]))
            for k in range(len(alpha))]

    lay0 = _direction_layout(er, ec)
    lay1 = _direction_layout(ec, er)
    t0 = _prep_direction(er, ec, ev, lay0)
    t1 = _prep_direction(ec, er, ev, lay1)

    x0o = np.zeros((NODES_PAD, D), dtype=np.float32)
    x0i = np.zeros((NODES_PAD, D), dtype=np.float32)
    x0o[:N_NODES] = np.maximum(od, 1e-8)[:, None] * H
    x0i[:N_NODES] = np.maximum(idg, 1e-8)[:, None] * H

    hpad = np.zeros((NODES_PAD, D), dtype=np.float32)
    hpad[:N_NODES] = H
    ident = np.eye(128, dtype=np.float32)
    theta = np.ascontiguousarray(np.asarray(Theta, dtype=np.float32)).astype(
        bfloat16)

    nsmax = 0
    nemax = 0
    for lay in (lay0, lay1):
        nb = {}
        ne = {}
        for (bi, c), v in lay["nreal"].items():
            nb[bi] = nb.get(bi, 0) + v
        for (bi, c), v in lay["next"].items():
            ne[bi] = ne.get(bi, 0) + v
        lay["nblk"] = nb
        lay["neblk"] = ne
        nsmax = max(nsmax, max(nb.values()))
        nemax = max(nemax, max(ne.values()))
    iota = np.tile(np.arange(128, dtype=np.float32), 1)[None, :].repeat(
        128, axis=0).astype(bfloat16)

    in_maps = []
    for m in range(NCORES):
        in_maps.append({
            "x0_out": x0o,
            "x0_in": x0i,
            "hfm": np.ascontiguousarray(hpad[m * SHARD:(m + 1) * SHARD].T),
            "theta": theta,
            "ident": ident,
            "iota": iota,
            "idx0": t0[m]["idx"],
            "denc0": t0[m]["denc"].astype(bfloat16),
            "val0": t0[m]["val"],
            "idx1": t1[m]["idx"],
            "denc1": t1[m]["denc"].astype(bfloat16),
            "val1": t1[m]["val"],
        })
    meta = {"coef": coef, "lay": [lay0, lay1], "nsmax": int(nsmax),
            "nemax": int(nemax)}
    return in_maps, meta


def build_program(tc, ins, outs, meta):
    """Emit the full SPMD program into TileContext tc."""
    import concourse.mybir as mybir

    nc = tc.nc
    f32 = mybir.dt.float32
    f32r = mybir.dt.float32r
    bf16 = mybir.dt.bfloat16
    i16 = mybir.dt.int16
    EQ, MUL, ADD = (mybir.AluOpType.is_equal, mybir.AluOpType.mult,
                    mybir.AluOpType.add)

    coef = meta["coef"]
    nsmax = meta["nsmax"]
    nemax = meta["nemax"]
    lays = meta["lay"]
    rg = [list(range(NCORES))]

    bounce = [nc.dram_tensor(f"bounce{d}", [SHARD, D], f32r,
                             kind="Internal") for d in range(2)]
    xbuf = [[nc.dram_tensor(f"xbuf{d}_{p}", [NODES_PAD, D], f32r,
                            kind="Internal", addr_space="Shared")
             for p in range(2)] for d in range(2)]

    tabs = [
        (ins["idx0"], ins["denc0"], ins["val0"]),
        (ins["idx1"], ins["denc1"], ins["val1"]),
    ]
    x0 = [ins["x0_out"], ins["x0_in"]]

    with (
        tc.tile_pool(name="const", bufs=1) as cpool,
        tc.tile_pool(name="work", bufs=1) as wpool,
        tc.tile_pool(name="stream", bufs=2) as spool,
        tc.tile_pool(name="smat", bufs=2) as spool2,
        tc.tile_pool(name="xc", bufs=4) as xpool,
        tc.tile_pool(name="fin", bufs=1) as fpool,
        tc.tile_pool(name="ps", bufs=4, space="PSUM") as pspool,
        tc.tile_pool(name="psf", bufs=2, space="PSUM") as psfpool,
    ):
        iota_s = cpool.tile([128, 128], bf16, tag="iota")
        nc.sync.dma_start(iota_s[:], ins["iota"][:])
        ident_s = cpool.tile([128, 128], f32, tag="ident")
        nc.sync.dma_start(ident_s[:], ins["ident"][:])
        theta_s = cpool.tile([64, D], bf16, tag="theta")
        nc.sync.dma_start(theta_s[:], ins["theta"][:])

        # resident per-direction idx / denc (bf16, ext) / val (f32) tables
        denc_res = []
        val_res = []
        idx_res = []
        for d in range(2):
            it_ = wpool.tile([128, lays[d]["nreal_tot"] * 8], i16,
                             tag=f"idx{d}")
            nc.sync.dma_start(it_[:], tabs[d][0][:])
            dt_ = wpool.tile([128, lays[d]["next_tot"]], bf16, tag=f"denc{d}")
            nc.sync.dma_start(dt_[:], tabs[d][1][:])
            vt_ = wpool.tile([128, lays[d]["nreal_tot"]], f32, tag=f"val{d}")
            nc.sync.dma_start(vt_[:], tabs[d][2][:])
            idx_res.append(it_)
            denc_res.append(dt_)
            val_res.append(vt_)

        st = wpool.tile([128, GROUPS, D], f32, tag="st")
        nc.vector.memset(st[:], 0.0)

        def emit_final(gs, gcnt):
            width = gcnt * 128
            stfm = fpool.tile([64, GPB * 128], bf16, tag="stfm")
            for j in range(gcnt):
                pt = psfpool.tile([64, 128], f32, tag="pt")
                nc.tensor.transpose(pt[:], st[:, gs + j, :], ident_s[:])
                nc.scalar.copy(out=stfm[:, j * 128:(j + 1) * 128], in_=pt[:])
            zp = psfpool.tile([64, GPB * 128], f32, tag="zp")
            nc.tensor.matmul(zp[:, :width], lhsT=theta_s[:],
                             rhs=stfm[:, :width], start=True, stop=True)
            sg = fpool.tile([64, GPB * 128], f32, tag="sg")
            nc.scalar.activation(sg[:, :width], zp[:, :width],
                                 mybir.ActivationFunctionType.Sigmoid)
            hf = fpool.tile([64, GPB * 128], f32, tag="hf")
            nc.sync.dma_start(
                hf[:, :width], ins["hfm"][:, gs * 128:gs * 128 + width])
            yt = fpool.tile([64, GPB * 128], f32, tag="yt")
            nc.vector.tensor_tensor(out=yt[:, :width], in0=sg[:, :width],
                                    in1=hf[:, :width], op=ADD)
            nc.sync.dma_start(
                outs["y"][:, gs * 128:gs * 128 + width], yt[:, :width])

        for hop in range(NUM_HOPS):
            for dirn in range(2):
                lay = lays[dirn]
                blocks = lay["blocks"]
                sched = lay["sched"]
                xsrc = (x0[dirn] if hop == 0
                        else xbuf[dirn][(hop - 1) % 2].ap())
                xv = xsrc.rearrange("(c n) d -> c n d", n=CHUNK)
                last = hop == NUM_HOPS - 1 and dirn == 1

                for bi, (g0, gc) in enumerate(blocks):
                    nb = lay["nblk"][bi]
                    neb = lay["neblk"][bi]
                    ob = lay["Oreal"][(bi, 0)]
                    oeb = lay["Oext"][(bi, 0)]
                    msgs = spool.tile([128, nsmax, D], f32r, tag="msgs")
                    for c in range(NCHUNKS):
                        ns = lay["nreal"][(bi, c)]
                        if ns == 0:
                            continue
                        o = lay["Oreal"][(bi, c)]
                        ol = o - ob
                        nc.gpsimd.dma_gather(
                            out_ap=msgs[:, ol:ol + ns, :],
                            in_ap=xv[c].bitcast(f32r),
                            idxs_ap=idx_res[dirn][:, o * 8:(o + ns) * 8],
                            num_idxs=ns * 128,
                            num_idxs_reg=ns * 128,
                            elem_size=D,
                            single_packet=False,
                            queue_num=c,
                        )
                    S = spool2.tile([128, nemax, 128], bf16, tag="S")
                    nc.vector.tensor_tensor(
                        out=S[:, :neb, :],
                        in0=iota_s[:].rearrange(
                            "p (o c) -> p o c", c=128).broadcast_to(
                            [128, neb, 128]),
                        in1=denc_res[dirn][:, oeb:oeb + neb].broadcast_to(
                            [128, neb, 128]),
                        op=EQ)
                    mbf = spool2.tile([128, nsmax, D], bf16, tag="mbf")
                    nc.vector.tensor_tensor(
                        out=mbf[:, :nb, :],
                        in0=msgs[:, :nb, :].bitcast(f32),
                        in1=val_res[dirn][:, ob:ob + nb].broadcast_to(
                            [128, nb, D]),
                        op=MUL)

                    for gl in range(gc):
                        g = g0 + gl
                        mms = sched[g]
                        if not mms:
                            continue
                        ps = pspool.tile([128, D], f32, tag="ps")
                        for j, (c, exl, rel) in enumerate(mms):
                            exb = lay["Oext"][(bi, c)] - oeb + exl
                            reb = lay["Oreal"][(bi, c)] - ob + rel
                            nc.tensor.matmul(
                                ps[:],
                                lhsT=S[:, exb, :],
                                rhs=mbf[:, reb, :],
                                start=(j == 0),
                                stop=(j == len(mms) - 1),
                            )
                        nc.vector.scalar_tensor_tensor(
                            out=st[:, g, :], in0=ps[:],
                            scalar=coef[hop][dirn], in1=st[:, g, :],
                            op0=MUL, op1=ADD)
                        if hop < NUM_HOPS - 1:
                            xc = xpool.tile([128, D], f32r, tag="xc")
                            nc.scalar.copy(out=xc[:], in_=ps[:])
                            bounce_v = bounce[dirn].ap().rearrange(
                                "(g p) f -> p g f", p=128)
                            nc.sync.dma_start(bounce_v[:, g, :], xc[:])
                    if last:
                        emit_final(g0, gc)

                if hop < NUM_HOPS - 1:
                    nc.gpsimd.collective_compute(
                        "AllGather", mybir.AluOpType.bypass,
                        replica_groups=rg,
                        ins=[bounce[dirn].ap().opt()],
                        outs=[xbuf[dirn][hop % 2].ap().opt()],
                    )


def kernel(**inputs) -> np.ndarray:
    return _run(inputs, trace=False)[0]


def kernel_traced(inputs, trace_kwargs=None):
    """Returns (output, BassKernelResults) with NTFF trace if available."""
    return _run(inputs, trace=True, trace_kwargs=trace_kwargs or {})


def _run(inputs, trace=False, trace_kwargs=None):
    import concourse.bacc as bacc
    import concourse.mybir as mybir
    import concourse.tile as tile
    from concourse.bass_utils import run_bass_kernel_spmd

    in_maps, meta = prep_host(**inputs)

    nc = bacc.Bacc("TRN2", target_bir_lowering=False, debug=False,
                   num_devices=NCORES, num_swdge_queues=4)
    f32 = mybir.dt.float32
    f32r = mybir.dt.float32r
    bf16 = mybir.dt.bfloat16
    i16 = mybir.dt.int16
    nsmax = meta["nsmax"]
    nemax = meta["nemax"]
    r0 = meta["lay"][0]["nreal_tot"]
    e0 = meta["lay"][0]["next_tot"]
    r1 = meta["lay"][1]["nreal_tot"]
    e1 = meta["lay"][1]["next_tot"]

    ins = {}
    shapes = {
        "x0_out": ([NODES_PAD, D], f32r),
        "x0_in": ([NODES_PAD, D], f32r),
        "hfm": ([D, SHARD], f32),
        "theta": ([D, D], bf16),
        "ident": ([128, 128], f32),
        "iota": ([128, 128], bf16),
        "idx0": ([128, r0 * 8], i16),
        "denc0": ([128, e0], bf16),
        "val0": ([128, r0], f32),
        "idx1": ([128, r1 * 8], i16),
        "denc1": ([128, e1], bf16),
        "val1": ([128, r1], f32),
    }
    for k, (shape, dt) in shapes.items():
        ins[k] = nc.dram_tensor(k, shape, dt, kind="ExternalInput").ap()
    y = nc.dram_tensor("y", [D, SHARD], f32, kind="ExternalOutput")

    with tile.TileContext(nc) as tc:
        build_program(tc, ins, {"y": y.ap()}, meta)
    nc.compile()

    kw = {}
    if trace:
        kw = dict(trace=True, trace_kwargs=trace_kwargs or {})
    res = run_bass_kernel_spmd(nc, in_maps, core_ids=list(range(NCORES)),
                               **kw)
    shards = [r["y"].T for r in res.results]  # each [SHARD, 64]
    out = np.concatenate(shards, axis=0)[:N_NODES]
    return np.ascontiguousarray(out.astype(np.float32)), res
